# revision 12
# baseline (speedup 1.0000x reference)
"""DifferentialAttention Trainium2 kernel (8-core SPMD), bf16 edition.

Sharding: 8 cores = 4 batches x 2 head-groups (8 heads each).
Each core computes, for its (batch, head-group):
  - x^T, weights DMA'd directly as bf16 (host pre-casts; no on-chip casts)
  - Q^T, K^T projections (bf16), V -> vaug bf16 with ones column;
    vaug2 = -lam*V (lambda folded in, ones col kept at 1)
  - per head: causal scores^T (row-packed s1/s2 in 512-col chunks, 256-col
    causal trim), exp on ACT writing one flat bf16 p tile per score, diag
    masks batched 4-blocks-at-a-time via gpsimd affine_select,
    u = [V|1]^T @ p (denominator folded into the matmul), softmax division
    via on-partition reciprocal + gpsimd partition_broadcast (no DRAM
    roundtrip, no PE broadcast), GroupNorm via bn_stats + cross-partition
    ones-matmul, ln/exp rsqrt trick
  - partial output = yn^T rows @ Wc[group rows] in bf16 (row-sharded
    c_proj), bf16 partial outputs summed on host (the "all-reduce after").
"""

import math
import sys

for _p in ("/opt/trn_rl_repo", "/root/.axon_site/_ro/trn_rl_repo"):
    if _p not in sys.path:
        sys.path.append(_p)

from contextlib import ExitStack

import numpy as np
import ml_dtypes

import concourse.mybir as mybir
import concourse.tile as tile
from concourse import bacc
from concourse.bass_utils import run_bass_kernel_spmd

F32 = mybir.dt.float32
F32R = mybir.dt.float32r
BF16 = mybir.dt.bfloat16
AF = mybir.ActivationFunctionType
OP = mybir.AluOpType

B, T, C = 4, 1024, 1024
NH = 16
HD = C // NH  # 64
NHL = 8  # heads per core
LAMBDA_INIT = 0.8 - 0.6 * math.exp(-0.3 * 1.0)
EPS = 1e-5
SCALE = 1.0 / math.sqrt(HD)
N_CORES = 8
NKT = T // 128  # 8 tk tiles
NKC = C // 128  # 8 contraction tiles

# flat p-tile column offsets: tiles 0-3 hold both 512-chunks (1024 cols),
# tiles 4-7 hold only chunk 1 (512 cols)
POFF = [1024 * i if i < 4 else 2048 + 512 * i for i in range(NKT)]
PW = POFF[7] + 512  # 6144


def _const(nc, val, shape):
    return nc.const_aps.tensor(val, shape)


def _bcast(dram_tile, parts, cols):
    import concourse.bass as bass

    ap = dram_tile[:]
    return bass.AP(tensor=ap.tensor, offset=ap.offset, ap=[[0, parts], [1, cols]])


def build_program(n_iters: int = 1, stop: str = "full"):
    nc = bacc.Bacc("TRN2", target_bir_lowering=False, debug=False)
    x_d = nc.dram_tensor("xbT", [C, T], BF16, kind="ExternalInput").ap()
    wq_d = nc.dram_tensor("wq", [C, 1024], BF16, kind="ExternalInput").ap()
    wk_d = nc.dram_tensor("wk", [C, 1024], BF16, kind="ExternalInput").ap()
    wv_d = nc.dram_tensor("wv", [C, 512], BF16, kind="ExternalInput").ap()
    wc_d = nc.dram_tensor("wc", [512, C], BF16, kind="ExternalInput").ap()
    neglam_d = nc.dram_tensor("neglam", [1, 64], F32, kind="ExternalInput").ap()
    out_d = nc.dram_tensor("outp", [T, C], BF16, kind="ExternalOutput").ap()

    with tile.TileContext(nc) as tc, ExitStack() as ctx:
        if n_iters == 1:
            _emit_iteration(nc, tc, x_d, wq_d, wk_d, wv_d, wc_d, neglam_d, out_d, stop)
        else:
            with tc.For_i(0, n_iters, 1):
                _emit_iteration(nc, tc, x_d, wq_d, wk_d, wv_d, wc_d, neglam_d, out_d, stop)

    nc.compile()
    return nc


def _diag_blocks_ap(pt, c):
    """AP over the 4 diagonal 128-blocks of chunk c in flat p tile pt."""
    import concourse.bass as bass

    base = pt[:]
    if c == 0:
        off, stride = 0, 1152  # blocks at 1152*i, i=0..3
    else:
        off, stride = 4096, 640  # blocks at 4096+640*(i-4), i=4..7
    return bass.AP(
        tensor=base.tensor,
        offset=base.offset + off,
        ap=[base.ap[0], [stride, 4], [1, 128]],
    )


def _emit_iteration(nc, tc, x_d, wq_d, wk_d, wv_d, wc_d, neglam_d, out_d, stop="full"):
    with ExitStack() as ctx:
        # ---------------- long-lived pools ----------------
        lp = ctx.enter_context(tc.tile_pool(name="long", bufs=1))
        qk = ctx.enter_context(tc.tile_pool(name="qk", bufs=1))

        # consts
        neglam_b = lp.tile([128, 1], F32, tag="neglam_b")
        nc.sync.dma_start(neglam_b[:], _bcast(neglam_d, 128, 1))
        ones64 = lp.tile([64, 64], F32, tag="ones64")
        nc.vector.tensor_copy(ones64[:], _const(nc, 1.0, (64, 64)))
        epsc = lp.tile([64, 1], F32, tag="epsc")
        nc.vector.memset(epsc[:], EPS)
        lnb = lp.tile([64, 1], F32, tag="lnb")
        nc.vector.memset(lnb[:], float(math.log(1.0 - LAMBDA_INIT)))

        # Vaug tiles: (128, 8 heads, 65) bf16, col 64 = ones
        vaug = [lp.tile([128, NHL, HD + 1], BF16, tag=f"vaug{t}", name=f"vaug{t}") for t in range(NKT)]
        vaug2 = [lp.tile([128, NHL, HD + 1], BF16, tag=f"vau2{t}", name=f"vau2{t}") for t in range(NKT)]
        # Q^T / K^T tiles bf16
        QT = [qk.tile([128, T], BF16, tag=f"qt{m}", name=f"qt{m}") for m in range(NKC)]
        KT = [qk.tile([128, T], BF16, tag=f"kt{m}", name=f"kt{m}") for m in range(NKC)]
        # c_proj weights, loaded during AB so phase E never waits on DMA
        wc_r = [lp.tile([128, C], BF16, tag=f"wc{k}", name=f"wcr{k}") for k in range(4)]

        # ---------------- phase A+B: loads + projections ----------------
        with ExitStack() as ab:
            wst = ab.enter_context(tc.tile_pool(name="wst", bufs=2))
            xtp = ab.enter_context(tc.tile_pool(name="xtp", bufs=1))
            psb = ab.enter_context(tc.tile_pool(name="psb", bufs=3, space="PSUM"))

            xT = [xtp.tile([128, T], BF16, tag=f"xt{k}", name=f"xt{k}") for k in range(NKC)]
            for cc in range(NKC):
                nc.sync.dma_start(xT[cc][:], x_d[cc * 128 : (cc + 1) * 128, :])

            def load_w(dram, k, width, tag):
                wr = wst.tile([128, width], BF16, tag=f"wr_{tag}_{k}", bufs=1)
                nc.sync.dma_start(wr[:], dram[k * 128 : (k + 1) * 128, :])
                return wr

            # --- Q^T projection (then K^T reusing weight slots)
            for name, dram, dest in (("q", wq_d, QT), ("k", wk_d, KT)):
                w_r = [load_w(dram, k, 1024, name) for k in range(NKC)]
                if name == "k":
                    for k in range(4):
                        nc.sync.dma_start(wc_r[k][:], wc_d[k * 128 : (k + 1) * 128, :])
                for m in range(NKC):
                    pq = psb.tile([128, T], F32, tag="proj", bufs=3)
                    for c0 in range(0, T, 512):
                        for k in range(NKC):
                            nc.tensor.matmul(
                                pq[:, c0 : c0 + 512],
                                w_r[k][:, m * 128 : (m + 1) * 128],
                                xT[k][:, c0 : c0 + 512],
                                start=(k == 0),
                                stop=(k == NKC - 1),
                            )
                    if m % 2 == 0:
                        nc.vector.tensor_copy(dest[m][:], pq[:])
                    else:
                        nc.scalar.copy(dest[m][:], pq[:])

            # --- V projection into vaug/vaug2 (ones columns hoisted)
            for tt in range(NKT):
                nc.gpsimd.memset(vaug[tt][:, :, HD : HD + 1], 1.0)
                nc.gpsimd.memset(vaug2[tt][:, :, HD : HD + 1], 1.0)
            wv_r = [load_w(wv_d, k, 512, "v") for k in range(NKC)]
            for tt in range(NKT):
                pv = psb.tile([128, 512], F32, tag="projv", bufs=2)
                for k in range(NKC):
                    nc.tensor.matmul(
                        pv[:],
                        xT[k][:, tt * 128 : (tt + 1) * 128],
                        wv_r[k][:],
                        start=(k == 0),
                        stop=(k == NKC - 1),
                    )
                pvr = pv[:].rearrange("p (h d) -> p h d", h=NHL)
                if tt % 2 == 0:
                    nc.vector.tensor_copy(vaug[tt][:, :, 0:HD], pvr)
                    nc.scalar.mul(vaug2[tt][:, :, 0:HD], pvr, neglam_b[:, 0:1])
                else:
                    nc.scalar.copy(vaug[tt][:, :, 0:HD], pvr)
                    nc.vector.tensor_scalar_mul(vaug2[tt][:, :, 0:HD], pvr, neglam_b[:, 0:1])

        if stop == "ab":
            for m in range(NKC):
                nc.sync.dma_start(out_d[m * 128 : (m + 1) * 128, :], QT[m][:])
            return

        # ---------------- phase C: attention per head ----------------
        yout = ctx.enter_context(tc.tile_pool(name="yn", bufs=1))
        with ExitStack() as cc_:
            pp = cc_.enter_context(tc.tile_pool(name="pp", bufs=1))
            yt = cc_.enter_context(tc.tile_pool(name="yt", bufs=1))
            sm = cc_.enter_context(tc.tile_pool(name="sm", bufs=2))
            pss = cc_.enter_context(tc.tile_pool(name="pss", bufs=2, space="PSUM"))
            psu = cc_.enter_context(tc.tile_pool(name="psu", bufs=2, space="PSUM"))

            yTn = [yout.tile([128, T], BF16, tag=f"ytn{k}", name=f"ytn{k}") for k in range(4)]

            meanAll = sm.tile([64, NHL], F32, tag="meanAll", bufs=1)
            varAll = sm.tile([64, NHL], F32, tag="varAll", bufs=1)
            yT_heads = {}
            nheads = NHL if not stop.startswith("c") else int(stop[1:])

            # flat persistent p tiles (zero-filled once; exp rewrites the
            # causal region every head, zeros below the diagonal persist)
            p1_all = pp.tile([128, PW], BF16, tag="p1", name="p1_all")
            p2_all = pp.tile([128, PW], BF16, tag="p2", name="p2_all")
            for i in range(NKT):
                d0 = i * 128 - (i // 4) * 512
                if d0 > 0:
                    nc.gpsimd.memset(p1_all[:, POFF[i] : POFF[i] + d0], 0.0)
                    nc.gpsimd.memset(p2_all[:, POFF[i] : POFF[i] + d0], 0.0)

            def emit_scores_chunk(j, c):
                """Scores+exp for head j, query chunk c (cols 512c..512c+512)."""
                q0 = c * 512
                ilast = min(NKT, (c + 1) * 4)
                for i in range(ilast):
                    c0 = (i // 4) * 512  # p-tile col base
                    lo = q0 - c0  # chunk base in p-tile coords
                    dc = max(0, i * 128 - q0)  # diag offset within chunk
                    cb = 256 if dc >= 256 else 0  # causal 256-granular trim
                    s1 = pss.tile([128, 512], F32, tag="s1", name=f"s1_{j}_{c}_{i}")
                    s2 = pss.tile([128, 512], F32, tag="s2", name=f"s2_{j}_{c}_{i}")
                    nc.tensor.matmul(
                        s1[:, cb:512],
                        KT[j][0:64, i * 128 : (i + 1) * 128],
                        QT[j][0:64, q0 + cb : q0 + 512],
                        start=True,
                        stop=True,
                        tile_position=(0, 0),
                    )
                    nc.tensor.matmul(
                        s2[:, cb:512],
                        KT[j][64:128, i * 128 : (i + 1) * 128],
                        QT[j][64:128, q0 + cb : q0 + 512],
                        start=True,
                        stop=True,
                        tile_position=(64, 0),
                    )
                    p1c = p1_all[:, POFF[i] + lo + dc : POFF[i] + lo + 512]
                    p2c = p2_all[:, POFF[i] + lo + dc : POFF[i] + lo + 512]
                    nc.scalar.activation(p1c, s1[:, dc:512], AF.Exp, scale=SCALE)
                    nc.scalar.activation(p2c, s2[:, dc:512], AF.Exp, scale=SCALE)
                # batched diagonal mask: 4 blocks per (score, chunk) at once
                for pt in (p1_all, p2_all):
                    nc.gpsimd.affine_select(
                        out=_diag_blocks_ap(pt, c),
                        in_=_diag_blocks_ap(pt, c),
                        compare_op=OP.is_ge,
                        fill=0.0,
                        base=0,
                        pattern=[[0, 4], [1, 128]],
                        channel_multiplier=-1,
                    )

            def emit_u_mms(j, c):
                """u matmuls for head j, chunk c. Returns (u1, u2) PSUM tiles."""
                ilast = min(NKT, (c + 1) * 4) - 1
                u1 = psu.tile([HD + 1, 512], F32, tag="u1", name=f"u1_{j}_{c}")
                u2 = psu.tile([HD + 1, 512], F32, tag="u2", name=f"u2_{j}_{c}")
                for i in range(ilast + 1):
                    lo = c * 512 - (i // 4) * 512
                    nc.tensor.matmul(
                        u1[:],
                        vaug[i][:, j, :],
                        p1_all[:, POFF[i] + lo : POFF[i] + lo + 512],
                        start=(i == 0),
                        stop=(i == ilast),
                    )
                for i in range(ilast + 1):
                    lo = c * 512 - (i // 4) * 512
                    nc.tensor.matmul(
                        u2[:],
                        vaug2[i][:, j, :],
                        p2_all[:, POFF[i] + lo : POFF[i] + lo + 512],
                        start=(i == 0),
                        stop=(i == ilast),
                    )
                return u1, u2

            def emit_recips(j, c, u1, u2):
                rr1 = sm.tile([1, 512], F32, tag="rr1", name=f"rr1_{j}_{c}")
                rr2 = sm.tile([1, 512], F32, tag="rr2", name=f"rr2_{j}_{c}")
                nc.vector.reciprocal(rr1[0:1, :], u1[64:65, :])
                nc.vector.reciprocal(rr2[0:1, :], u2[64:65, :])
                return rr1, rr2

            def emit_bcast(j, c, rr1, rr2):
                Rs = sm.tile([64, 1024], F32, tag="Rs", name=f"Rs_{j}_{c}")
                nc.gpsimd.partition_broadcast(Rs[:, 0:512], rr1[0:1, :])
                nc.gpsimd.partition_broadcast(Rs[:, 512:1024], rr2[0:1, :])
                return Rs

            def emit_combine(j, c, u1, u2, Rs):
                yT_h = yT_heads[j]
                t1 = sm.tile([64, 512], F32, tag="t1", name=f"t1_{j}_{c}")
                t2 = sm.tile([64, 512], F32, tag="t2", name=f"t2_{j}_{c}")
                nc.vector.tensor_tensor(t1[:], u1[0:HD, :], Rs[:, 0:512], OP.mult)
                nc.vector.tensor_tensor(t2[:], u2[0:HD, :], Rs[:, 512:1024], OP.mult)
                nc.vector.tensor_tensor(yT_h[:, c * 512 : (c + 1) * 512], t1[:], t2[:], OP.add)

            def emit_stats(j):
                yT_h = yT_heads[j]
                bstats = sm.tile([64, 2, 6], F32, tag="bst", name=f"bst_{j}")
                for si in range(2):
                    nc.vector.bn_stats(out=bstats[:, si, :], in_=yT_h[:, si * 512 : (si + 1) * 512])
                mv = sm.tile([64, 2], F32, tag="mv", name=f"mv_{j}")
                nc.vector.bn_aggr(out=mv[:], in_=bstats[:])
                st = sm.tile([64, 2], F32, tag="st", name=f"st_{j}")
                m2p = sm.tile([64, 1], F32, tag="m2p", name=f"m2p_{j}")
                nc.vector.tensor_tensor(m2p[:], mv[:, 0:1], mv[:, 0:1], OP.mult)
                nc.vector.tensor_tensor(st[:, 1:2], mv[:, 1:2], m2p[:], OP.add)
                nc.vector.tensor_copy(st[:, 0:1], mv[:, 0:1])
                pstat = psu.tile([64, 2], F32, tag="u1", name=f"pstat_{j}")
                nc.tensor.matmul(pstat[:], ones64[:], st[:], start=True, stop=True)
                stats = sm.tile([64, 2], F32, tag="stats", name=f"stats_{j}")
                nc.vector.tensor_scalar_mul(stats[:], pstat[:], 1.0 / 64.0)
                nc.vector.tensor_copy(meanAll[:, j : j + 1], stats[:, 0:1])
                m2 = sm.tile([64, 1], F32, tag="m2", name=f"m2_{j}")
                nc.vector.tensor_tensor(m2[:], stats[:, 0:1], stats[:, 0:1], OP.mult)
                nc.vector.tensor_tensor(varAll[:, j : j + 1], stats[:, 1:2], m2[:], OP.subtract)

            # head loop: per-chunk emission; cross-engine overlap comes from
            # the in-order engine queues (scores(j,1) fills PE while recips/
            # TTs of chunk 0 run on DVE/Pool).
            pend = None  # (j, c, u1, u2, rr1, rr2) awaiting bcast+combine
            for j in range(nheads):
                yT_heads[j] = yt.tile([64, T], F32, tag=f"yT{j}", name=f"yTh{j}")
                for c in range(2):
                    emit_scores_chunk(j, c)
                    if pend is not None:
                        pj, pc, pu1, pu2, pr1, pr2 = pend
                        Rs = emit_bcast(pj, pc, pr1, pr2)
                        emit_combine(pj, pc, pu1, pu2, Rs)
                        if pc == 1:
                            emit_stats(pj)
                    u1, u2 = emit_u_mms(j, c)
                    rr1, rr2 = emit_recips(j, c, u1, u2)
                    pend = (j, c, u1, u2, rr1, rr2)
            pj, pc, pu1, pu2, pr1, pr2 = pend
            Rs = emit_bcast(pj, pc, pr1, pr2)
            emit_combine(pj, pc, pu1, pu2, Rs)
            emit_stats(pj)

            # ---- batched groupnorm: ln/exp once, then normalize all heads
            lnvAll = sm.tile([64, NHL], F32, tag="lnvAll", bufs=1)
            nc.scalar.activation(lnvAll[:, 0:nheads], varAll[:, 0:nheads], AF.Ln, bias=epsc[:])
            rstdAll = sm.tile([64, NHL], F32, tag="rstdAll", bufs=1)
            nc.scalar.activation(
                rstdAll[:, 0:nheads], lnvAll[:, 0:nheads], AF.Exp, scale=-0.5, bias=lnb[:]
            )
            for j in range(nheads):
                if j % 2 == 0:
                    nc.vector.tensor_scalar(
                        out=yTn[j // 2][0:64, :],
                        in0=yT_heads[j][:],
                        scalar1=meanAll[:, j : j + 1],
                        scalar2=rstdAll[:, j : j + 1],
                        op0=OP.subtract,
                        op1=OP.mult,
                    )
                else:
                    ymv = sm.tile([64, T], BF16, tag="ymv", bufs=1)
                    nc.vector.tensor_scalar(
                        out=ymv[:],
                        in0=yT_heads[j][:],
                        scalar1=meanAll[:, j : j + 1],
                        scalar2=rstdAll[:, j : j + 1],
                        op0=OP.subtract,
                        op1=OP.mult,
                    )
                    nc.sync.dma_start(yTn[j // 2][64:128, :], ymv[:])

            if stop.startswith("c"):
                for k in range(nheads // 2):
                    nc.sync.dma_start(
                        out_d[k * 128 : (k + 1) * 128, :], yTn[k][:]
                    )
                return

        # ---------------- phase E: output projection ----------------
        with ExitStack() as ee:
            oe = ee.enter_context(tc.tile_pool(name="oe", bufs=2))
            pso = ee.enter_context(tc.tile_pool(name="pso", bufs=3, space="PSUM"))
            for m in range(NKC):
                po = pso.tile([128, C], F32, tag="o")
                for c0 in range(0, C, 512):
                    for kk in range(4):
                        nc.tensor.matmul(
                            po[:, c0 : c0 + 512],
                            yTn[kk][:, m * 128 : (m + 1) * 128],
                            wc_r[kk][:, c0 : c0 + 512],
                            start=(kk == 0),
                            stop=(kk == 3),
                        )
                osb = oe.tile([128, C], BF16, tag="osb")
                if m % 2 == 0:
                    nc.vector.tensor_copy(osb[:], po[:])
                else:
                    nc.scalar.copy(osb[:], po[:])
                nc.sync.dma_start(out_d[m * 128 : (m + 1) * 128, :], osb[:])


_PROGRAM_CACHE = {}


def get_program(n_iters: int = 1):
    if n_iters not in _PROGRAM_CACHE:
        _PROGRAM_CACHE[n_iters] = build_program(n_iters)
    return _PROGRAM_CACHE[n_iters]


def make_in_maps(x, Wq, Wk, Wv, Wc, lambda_q1, lambda_k1, lambda_q2, lambda_k2):
    lam = (
        math.exp(float(np.sum(lambda_q1.astype(np.float64) * lambda_k1.astype(np.float64))))
        - math.exp(float(np.sum(lambda_q2.astype(np.float64) * lambda_k2.astype(np.float64))))
        + LAMBDA_INIT
    )
    bf = ml_dtypes.bfloat16
    neglam = np.full((1, 64), -lam, dtype=np.float32)
    in_maps = []
    for core in range(N_CORES):
        b, g = core // 2, core % 2
        in_maps.append(
            {
                "xbT": np.ascontiguousarray(x[b].T).astype(bf),
                "wq": np.ascontiguousarray(Wq[:, g * 1024 : (g + 1) * 1024]).astype(bf),
                "wk": np.ascontiguousarray(Wk[:, g * 1024 : (g + 1) * 1024]).astype(bf),
                "wv": np.ascontiguousarray(Wv[:, g * 512 : (g + 1) * 512]).astype(bf),
                "wc": np.ascontiguousarray(Wc[g * 512 : (g + 1) * 512, :]).astype(bf),
                "neglam": neglam,
            }
        )
    return in_maps


def kernel(x, Wq, Wk, Wv, Wc, lambda_q1, lambda_k1, lambda_q2, lambda_k2):
    x = np.asarray(x, dtype=np.float32)
    in_maps = make_in_maps(
        x,
        np.asarray(Wq, np.float32),
        np.asarray(Wk, np.float32),
        np.asarray(Wv, np.float32),
        np.asarray(Wc, np.float32),
        np.asarray(lambda_q1, np.float32),
        np.asarray(lambda_k1, np.float32),
        np.asarray(lambda_q2, np.float32),
        np.asarray(lambda_k2, np.float32),
    )
    nc = get_program(1)
    res = run_bass_kernel_spmd(nc, in_maps, list(range(N_CORES)))
    out = np.empty((B, T, C), dtype=np.float32)
    for b in range(B):
        out[b] = res.results[2 * b]["outp"].astype(np.float32) + res.results[
            2 * b + 1
        ]["outp"].astype(np.float32)
    return out
